# revision 8
# baseline (speedup 1.0000x reference)
"""Trainium2 kernel for nn_ApproxMultLayer.

The reference quantizes x[32,256] and w[256,256] to uint8, applies an
approximate 8x8-bit multiplier circuit elementwise and reduces over the
inner dim: acc[b,o] = sum_i T[xq[b,i], wq[o,i]], out = acc / 255^2.

The approximate multiplier's final stage is an exact 16-bit ripple add of
four sub-products, each a function of one (x-nibble, w-nibble) pair:

    T[a,b] = 256*F1[ah,bh] + 16*F[al,bh] + 16*F[ah,bl] + F[al,bl]

(verified exhaustively; the sum never reaches 2^16, so no wraparound).
F1/F are 16x16 tables. Each term sum_i Fk[xnib[b,i], wnib[o,i]] is a
matmul between a host-gathered x-side table and a one-hot encoding of the
w nibble:

    acc[b,o] = sum_{i,v} L[(i,v), b] * onehot(wnib[o,i])[v]

with contraction K = 256 i * 16 v * 2 nibble-sides = 8192, and the two
scale-plane pairs folded into the stationary M dim (M = 2*32 = 64).
All table values are 8-bit ints times powers of two => exact in bf16;
PSUM accumulates in fp32 (partials < 2^24) => bit-exact result.

Sharding: contraction (K) split across the 8 cores, 1024 each; each core
emits a [32,256] fp32 integer-valued partial; host sums and scales.
"""

import numpy as np
import ml_dtypes


def _ensure_ntff_hook():
    """bass_utils imports antenv.axon_hooks when trace=True under axon;
    some images lack that module. Provide it (and register the ctypes
    hook the boot shim would have registered) so tracing works instead
    of crashing."""
    import importlib
    import sys
    import types

    try:
        hooks = importlib.import_module("antenv.axon_hooks")
    except ImportError:
        hooks = types.ModuleType("antenv.axon_hooks")
        hooks._axon_ntff_profile_hook = None

        def set_axon_ntff_profile_hook(h, _m=hooks):
            _m._axon_ntff_profile_hook = h

        def get_axon_ntff_profile_hook(_m=hooks):
            return _m._axon_ntff_profile_hook

        hooks.set_axon_ntff_profile_hook = set_axon_ntff_profile_hook
        hooks.get_axon_ntff_profile_hook = get_axon_ntff_profile_hook
        sys.modules["antenv.axon_hooks"] = hooks

    if hooks.get_axon_ntff_profile_hook() is None:
        try:
            from trn_agent_boot.trn_boot import _ntff_profile_via_ctypes

            hook = _ntff_profile_via_ctypes("/opt/axon/libaxon_pjrt.so")
            if hook is not None:
                hooks.set_axon_ntff_profile_hook(hook)
        except Exception:
            pass  # tracing degrades; compile + run still work


_ensure_ntff_hook()

SCALE = 255.0
B, IN, OUT = 32, 256, 256
N_CORES = 8
K_PER_CORE = 1024  # 2 nibble-sides * 32 i * 16 v
KT = K_PER_CORE // 128  # 8 k-tiles of 128


# ---------------------------------------------------------------------------
# Approximate-multiplier nibble tables (numpy re-implementation of the circuit)
# ---------------------------------------------------------------------------

def _badd4(a, b, c, d, cin):
    t = a + b + c + d + cin
    return t // 2, t % 2


def _badd2(a, b, cin):
    t = a + b + cin
    return t // 2, t % 2


def _grid4(Ab, Bb):
    G = [[0] * 8 for _ in range(4)]
    for r in range(4):
        for k in range(4):
            G[r][(4 - r) + k] = Ab[k] & Bb[3 - r]
    return G


def _reduce4(G):
    R = [0] * 8
    R[7] = G[0][7] | G[1][7] | G[2][7] | G[3][7]
    R[6] = G[0][6] | G[1][6] | G[2][6] | G[3][6]
    p1 = G[0][5] ^ G[1][5]
    p2 = G[2][5] ^ G[3][5]
    R[5] = p1 ^ p2
    carry = (p1 & p2) | (G[0][5] & G[1][5])
    R[4] = G[0][4] ^ G[1][4] ^ G[2][4] ^ G[3][4] ^ carry
    c = 0
    for col in (3, 2, 1, 0):
        c, R[col] = _badd4(G[0][col], G[1][col], G[2][col], G[3][col], c)
    return R


def _two_row(aH, aL, bH, bL, c0):
    row0 = {c0: aH & bL, c0 + 1: aL & bL}
    row1 = {c0 - 1: aH & bH, c0: aL & bH}
    R = [0] * 8
    c = 0
    for col in (c0 + 1, c0, c0 - 1, c0 - 2):
        c, R[col] = _badd2(row0.get(col, 0), row1.get(col, 0), c)
    return R


def _val8(R):
    v = 0
    for i in range(8):
        v = v + (R[i] << (7 - i))
    return v


def _build_tables():
    n = np.arange(16, dtype=np.int64)
    x, y = n[:, None], n[None, :]
    xb = [(x >> (3 - i)) & 1 for i in range(4)]
    yb = [(y >> (3 - i)) & 1 for i in range(4)]

    # F: approximate 4x4 product (reduce4 of the partial-product grid)
    F = _val8(_reduce4(_grid4(xb, yb)))

    # F1: the R1 term -- 2x2-bit sub-products of the two high nibbles,
    # combined with an exact 4-input ripple.
    HH = _two_row(xb[0], xb[1], yb[0], yb[1], 2)
    HL = _two_row(xb[0], xb[1], yb[2], yb[3], 4)
    LH = _two_row(xb[2], xb[3], yb[0], yb[1], 4)
    LL = _two_row(xb[2], xb[3], yb[2], yb[3], 6)
    c = 0
    R1 = [0] * 8
    for col in range(7, -1, -1):
        c, R1[col] = _badd4(HH[col], LH[col], HL[col], LL[col], c)
    F1 = _val8(R1)
    return F1, F


_F1, _F = _build_tables()


# ---------------------------------------------------------------------------
# Bass program (built once; same NEFF on all 8 cores)
# ---------------------------------------------------------------------------

_BASS_CACHE = {}


def _get_bass():
    if "nc" in _BASS_CACHE:
        return _BASS_CACHE["nc"]
    import concourse.bass as bass
    import concourse.mybir as mybir

    nc = bass.Bass()
    # single fused input: per k-tile, cols 0:64 = lhsT planes, 64:320 = one-hot
    inp = nc.declare_dram_parameter(
        "inp", [128, KT, 64 + OUT], mybir.dt.bfloat16, isOutput=False
    )
    out = nc.declare_dram_parameter(
        "out", [64, OUT], mybir.dt.float32, isOutput=True
    )

    H = KT // 2
    with (
        nc.sbuf_tensor([128, KT, 64 + OUT], mybir.dt.bfloat16) as it,
        nc.sbuf_tensor([64, OUT], mybir.dt.float32) as osb,
        nc.psum_tensor([64, OUT], mybir.dt.float32) as psum,
        nc.semaphore("dsem") as dsem,
        nc.semaphore("psem") as psem,
        nc.semaphore("vsem") as vsem,
        nc.Block() as block,
    ):

        @block.sync
        def _(sync):
            sync.dma_start(it[:, 0:H], inp[:, 0:H]).then_inc(dsem, 16)
            sync.dma_start(it[:, H:KT], inp[:, H:KT]).then_inc(dsem, 16)
            sync.wait_ge(vsem, 1)
            sync.dma_start(out[:], osb[:]).then_inc(dsem, 16)
            sync.wait_ge(dsem, 48)

        @block.tensor
        def _(tensor):
            tensor.wait_ge(dsem, 16)
            for t in range(H):
                nc.tensor.matmul(
                    psum[:],
                    lhsT=it[:, t, 0:64],
                    rhs=it[:, t, 64 : 64 + OUT],
                    start=(t == 0),
                    stop=False,
                )
            tensor.wait_ge(dsem, 32)
            for t in range(H, KT):
                mm = nc.tensor.matmul(
                    psum[:],
                    lhsT=it[:, t, 0:64],
                    rhs=it[:, t, 64 : 64 + OUT],
                    start=False,
                    stop=(t == KT - 1),
                )
            mm.then_inc(psem, 1)

        @block.vector
        def _(vector):
            vector.wait_ge(psem, 1)
            nc.vector.tensor_copy(osb[:], psum[:]).then_inc(vsem, 1)

    _BASS_CACHE["nc"] = nc
    return nc


# ---------------------------------------------------------------------------
# Host-side prep + launch
# ---------------------------------------------------------------------------

last_results = None  # BassKernelResults of the most recent launch (for profiling)


def _quantize(v):
    # match jnp: f32 multiply, round-half-even, clip
    vq = np.clip(np.round(v.astype(np.float32) * np.float32(SCALE)), 0.0, 255.0)
    return vq.astype(np.int64)


def kernel(x, w):
    from concourse.bass_utils import run_bass_kernel_spmd

    x = np.asarray(x)
    w = np.asarray(w)
    xq = _quantize(x)  # [B, IN]
    wq = _quantize(w)  # [OUT, IN]
    xh, xl = xq >> 4, xq & 15
    wh, wl = wq >> 4, wq & 15

    bf16 = ml_dtypes.bfloat16

    # x-side gathered tables, [(i,v), plane*b] -- values exact in bf16
    # plane scales 256/16/16/1 are folded in (power-of-two => exact).
    P1 = (256.0 * _F1.astype(np.float32))[xh, :]  # [B, IN, 16] (b,i,v)
    P2 = (16.0 * _F.astype(np.float32))[xl, :]
    P3 = (16.0 * _F.astype(np.float32))[xh, :]
    P4 = (1.0 * _F.astype(np.float32))[xl, :]

    def to_kv_b(t):  # [B, IN, 16] -> [IN*16, B]
        return t.transpose(1, 2, 0).reshape(IN * 16, B)

    LA = np.concatenate([to_kv_b(P1), to_kv_b(P2)], axis=1)  # [4096, 64]
    LB = np.concatenate([to_kv_b(P3), to_kv_b(P4)], axis=1)  # [4096, 64]

    # one-hot of w nibbles, [(i,v), o]
    v_iota = np.arange(16, dtype=np.int64)[None, :, None]
    RA = (wh.T[:, None, :] == v_iota).reshape(IN * 16, OUT).astype(np.float32)
    RB = (wl.T[:, None, :] == v_iota).reshape(IN * 16, OUT).astype(np.float32)

    # shard the contraction: core c takes i in [32c, 32c+32) for both
    # nibble-sides => local K = 512 + 512 = 1024 = KT tiles of 128.
    in_maps = []
    for c in range(N_CORES):
        rows = slice(512 * c, 512 * (c + 1))
        L = np.concatenate([LA[rows], LB[rows]], axis=0)  # [1024, 64]
        R = np.concatenate([RA[rows], RB[rows]], axis=0)  # [1024, 256]
        F = np.concatenate([L, R], axis=1)  # [1024, 320]
        # [K, m] -> [p, t, m] with K = 128*t + p
        F = F.reshape(KT, 128, 64 + OUT).transpose(1, 0, 2)
        in_maps.append({"inp": np.ascontiguousarray(F).astype(bf16)})

    nc = _get_bass()
    res = run_bass_kernel_spmd(nc, in_maps, core_ids=list(range(N_CORES)))
    global last_results
    last_results = res

    acc = np.zeros((B, OUT), dtype=np.float64)
    for c in range(N_CORES):
        part = res.results[c]["out"].astype(np.float64)  # [64, OUT]
        acc += part[0:B] + part[B : 2 * B]

    # match reference arithmetic: int32 acc -> float32, then fp32 divide
    return acc.astype(np.float32) / np.float32(SCALE * SCALE)
